# revision 3
# baseline (speedup 1.0000x reference)
"""Channel-attention (CAM) Bass kernel for TRN2, SPMD over 8 NeuronCores.

Computes, for each batch b:
    A   = inputs[b].reshape(HW, C)
    G   = A.T @ A                      (Gram, [C, C])
    S   = softmax(G, axis=-1)
    out = gamma * (A @ S) + A

Sharding: data-parallel over batch. 16 batches / 8 cores = 2 batches per core.

Numerics: residual form
    out = A @ (gamma*S - gamma*I) + (1 + gamma) * A
The (1+gamma)*A term is applied from the exact fp32 copy of A in the DVE
epilogue, so matmul precision only touches the gamma*(S - I) term.  That
lets both matmuls run in fp8e4 with DoubleRow perf mode (2 contraction
rows per PE cell per cycle).

Per-core schedule (per batch):
  - A loaded fp32 as 8 DMA groups of [128, 4, 512]; ALL input-load DMAs
    (both batches) are dispatched before any output DMA so input gets
    queue priority on the shared DMA engines.
  - GpSimd casts each group to fp8 (the raw fp32 tile stays resident for
    the epilogue residual).
  - Gram: fp8 DoubleRow matmuls (2 chunks of 128 rows per MM) into 4 PSUM
    banks; PE transposes of A (fp8, for the attend stationary operand)
    interleaved per group, last 2 groups deferred to cover softmax.
  - Softmax: DVE row-max (negated) -> ScalarE Exp with accum_out row-sum
    -> DVE reciprocal -> scale by gamma -> S'' = (E * gamma*r) - gamma*I
    written as fp8 into pair-layout tiles [128, 2, 512].
  - Attend: per row chunk, 2 DoubleRow MMs (256 channels each).
  - Epilogue: out = psum + (1+gamma)*raw in one DVE scalar_tensor_tensor,
    then DMA out per 2-chunk group.
  - ~36 warmup matmuls at kernel start push the PE HAM clock-gate to
    8/8 (2.4 GHz) before the first Gram work arrives.
"""

import numpy as np

import concourse.bass as bass
import concourse.mybir as mybir
import concourse.tile as tile
from concourse import bacc
from concourse.bass import ds, ts
from concourse.masks import make_identity

P = 128
N_CORES = 8
B_TOTAL = 16
B_PER_CORE = B_TOTAL // N_CORES  # 2
H = 64
W = 64
HW = H * W          # 4096
C = 512
KO = HW // P        # 32 row chunks of A
M = C // P          # 4 channel chunks
NQ = M // 2         # 2 channel-block pairs (DoubleRow)
NG = 8              # DMA groups
KPG = KO // NG      # chunks per group (4)
DEFER_G = 2         # transpose groups deferred past the Gram to cover softmax
N_WARM = 36         # warmup matmuls (~3.6us) to flip HAM to 8/8

F32 = mybir.dt.float32
BF16 = mybir.dt.bfloat16
F8 = mybir.dt.float8e4
AX = mybir.AxisListType
ALU = mybir.AluOpType
ACT_FN = mybir.ActivationFunctionType
DR = mybir.MatmulPerfMode.DoubleRow


def _build_kernel(tc, a_dram, gamma_dram, o_dram):
    nc = tc.nc
    from contextlib import ExitStack

    with ExitStack() as ctx:
        const_pool = ctx.enter_context(tc.tile_pool(name="const", bufs=1))
        a_pool = ctx.enter_context(tc.tile_pool(name="a", bufs=13))
        a8_pool = ctx.enter_context(tc.tile_pool(name="a8", bufs=10))
        at_pool = ctx.enter_context(tc.tile_pool(name="at", bufs=2 * NQ * NG))
        e_pool = ctx.enter_context(tc.tile_pool(name="e", bufs=2))
        s_pool = ctx.enter_context(tc.tile_pool(name="s", bufs=2 * NQ))
        st_pool = ctx.enter_context(tc.tile_pool(name="st", bufs=16))
        o_pool = ctx.enter_context(tc.tile_pool(name="o", bufs=3))
        pg_pool = ctx.enter_context(tc.tile_pool(name="pg", bufs=M, space="PSUM"))
        pt_pool = ctx.enter_context(tc.tile_pool(name="pt", bufs=2, space="PSUM"))
        po_pool = ctx.enter_context(tc.tile_pool(name="po", bufs=2, space="PSUM"))

        # ---- input DMAs first: gets them to the head of every DMA queue ----
        gamma_sb = const_pool.tile([P, 1], F32, tag="gamma")
        nc.sync.dma_start(gamma_sb, gamma_dram)
        raws = [[None] * NG for _ in range(B_PER_CORE)]
        for b in range(B_PER_CORE):
            a_b = a_dram[b].rearrange("(ko p) c -> p ko c", p=P)
            for g in range(NG):
                raw = a_pool.tile([P, KPG, C], F32, tag="a", name="raw")
                nc.sync.dma_start(raw, a_b[:, ts(g, KPG), :])
                raws[b][g] = raw

        # ---- constants ----
        ident8 = const_pool.tile([P, P], F8, tag="ident8")
        make_identity(nc, ident8)
        gamma2_sb = const_pool.tile([P, 1], F32, tag="gamma2")
        nc.vector.tensor_scalar_add(gamma2_sb, gamma_sb, 1.0)
        # identrow[m]: gamma * I placed at columns [128m, 128m+128) of a
        # [128, 512] row block, fp32
        identrow = []
        for m in range(M):
            ir = const_pool.tile([P, C], F32, tag=f"identrow{m}", name="ir")
            nc.gpsimd.memset(ir, 0.0)
            make_identity(nc, ir[:, ts(m, P)], nomemset=True)
            nc.vector.tensor_scalar_mul(ir, ir, gamma_sb)
            identrow.append(ir)

        # ---- PE warmup: ~3.6us of back-to-back matmuls flips HAM to 8/8
        # before the first Gram group lands. ----
        warm_ps = pg_pool.tile([P, P], F32, tag="pg", name="warm")
        for _ in range(N_WARM):
            nc.tensor.matmul(warm_ps, ident8, ident8, start=True, stop=True)

        o_bs = [o_dram[b].rearrange("(u p) c -> p u c", p=P) for b in range(B_PER_CORE)]

        for b in range(B_PER_CORE):
            a8 = []
            at = [[None] * NG for _ in range(NQ)]
            g_ps = [pg_pool.tile([P, C], F32, tag="pg", name="g_ps") for _ in range(M)]

            def do_transposes(g, b=b, a8=a8, at=at):
                for q in range(NQ):
                    atg = at_pool.tile([P, 2, KPG * P], F8, tag="at", name="at")
                    for i in range(2):
                        m = 2 * q + i
                        # fp8 transpose-mode requires output element step 2
                        pt = pt_pool.tile([P, KPG * P, 2], F8, tag="pt", name="pt")
                        for j in range(KPG):
                            nc.tensor.transpose(
                                pt[:, ts(j, P), 0],
                                a8[g][:, j, ts(m, P)],
                                ident8,
                            )
                        nc.scalar.activation(
                            atg[:, i, :], pt[:, :, 0], ACT_FN.Copy, bias=0.0, scale=1.0
                        )
                    at[q][g] = atg

            # Gram accumulation (fp8 DoubleRow, 2 chunks per MM); transposes
            # interleaved per group except the last DEFER_G groups (those
            # cover the softmax latency)
            for g in range(NG):
                a8g = a8_pool.tile([P, KPG, C], F8, tag="a8", name="a8g")
                nc.gpsimd.tensor_copy(out=a8g, in_=raws[b][g])
                a8.append(a8g)
                if g < NG - 1:
                    for t in range(KPG // 2):
                        pair = a8g[:, 2 * t : 2 * t + 2, :]
                        for m in range(M):
                            nc.tensor.matmul(
                                g_ps[m],
                                pair[:, :, ts(m, P)],
                                pair,
                                start=(g == 0 and t == 0),
                                stop=False,
                                perf_mode=DR,
                            )
                    if g < NG - DEFER_G:
                        do_transposes(g)
            # last group m-outer: G[m] completes in m order so the softmax
            # chain pipelines per-m right behind the Gram
            gl = NG - 1
            for m in range(M):
                for t in range(KPG // 2):
                    pair = a8[gl][:, 2 * t : 2 * t + 2, :]
                    nc.tensor.matmul(
                        g_ps[m],
                        pair[:, :, ts(m, P)],
                        pair,
                        start=False,
                        stop=(t == KPG // 2 - 1),
                        perf_mode=DR,
                    )

            # Row softmax of G -> S'' = gamma*S - gamma*I, fp8 pair layout.
            s_pair = [
                s_pool.tile([P, 2, C], F8, tag="s", name="s_pair") for _ in range(NQ)
            ]
            for m in range(M):
                negmax = st_pool.tile([P, 1], F32, tag="stat", name="negmax")
                nc.vector.tensor_reduce(
                    negmax, g_ps[m], axis=AX.X, op=ALU.max, negate=True
                )
                e = e_pool.tile([P, C], F32, tag="e", name="e")
                dsum = st_pool.tile([P, 1], F32, tag="stat", name="dsum")
                nc.scalar.activation(
                    e, g_ps[m], ACT_FN.Exp, bias=negmax, scale=1.0, accum_out=dsum
                )
                r = st_pool.tile([P, 1], F32, tag="stat", name="r")
                nc.vector.reciprocal(r, dsum)
                r2 = st_pool.tile([P, 1], F32, tag="stat", name="r2")
                nc.vector.tensor_scalar_mul(r2, r, gamma_sb)
                nc.vector.scalar_tensor_tensor(
                    s_pair[m // 2][:, m % 2, :],
                    e,
                    r2,
                    identrow[m],
                    op0=ALU.mult,
                    op1=ALU.subtract,
                )

            # deferred transposes run on PE while softmax latency drains
            for g in range(NG - DEFER_G, NG):
                do_transposes(g)

            # Attend (A @ S'') + residual epilogue; out-DMA per 2 chunks
            o_sb = None
            for t_i in range(KO):
                g, j = t_i // KPG, t_i % KPG
                o_ps = po_pool.tile([P, C], F32, tag="po", name="o_ps")
                for q in range(NQ):
                    nc.tensor.matmul(
                        o_ps,
                        at[q][g][:, :, ts(j, P)],
                        s_pair[q],
                        start=(q == 0),
                        stop=(q == NQ - 1),
                        perf_mode=DR,
                    )
                if t_i % 2 == 0:
                    o_sb = o_pool.tile([P, 2, C], F32, tag="o", name="o_sb")
                nc.vector.scalar_tensor_tensor(
                    o_sb[:, t_i % 2, :],
                    raws[b][g][:, j, :],
                    gamma2_sb,
                    o_ps,
                    op0=ALU.mult,
                    op1=ALU.add,
                )
                if t_i % 2 == 1:
                    nc.sync.dma_start(o_bs[b][:, t_i - 1 : t_i + 1, :], o_sb)


_NC_CACHE = None


def build():
    global _NC_CACHE
    if _NC_CACHE is not None:
        return _NC_CACHE
    nc = bacc.Bacc(
        "TRN2",
        target_bir_lowering=False,
        debug=False,
        enable_asserts=False,
        num_devices=N_CORES,
    )
    a_dram = nc.dram_tensor("a", [B_PER_CORE, HW, C], F32, kind="ExternalInput").ap()
    gamma_dram = nc.dram_tensor("gamma", [P, 1], F32, kind="ExternalInput").ap()
    o_dram = nc.dram_tensor("o", [B_PER_CORE, HW, C], F32, kind="ExternalOutput").ap()
    with tile.TileContext(nc) as tc:
        _build_kernel(tc, a_dram, gamma_dram, o_dram)
    nc.compile()
    _NC_CACHE = nc
    return nc


def make_in_maps(inputs, gamma):
    x = np.ascontiguousarray(np.asarray(inputs, dtype=np.float32)).reshape(
        B_TOTAL, HW, C
    )
    gb = np.ascontiguousarray(
        np.broadcast_to(np.asarray(gamma, dtype=np.float32).reshape(1, 1), (P, 1))
    )
    return [
        {"a": x[i * B_PER_CORE : (i + 1) * B_PER_CORE], "gamma": gb}
        for i in range(N_CORES)
    ]


def run(inputs, gamma, trace=False, **kw):
    from concourse import bass_utils

    nc = build()
    in_maps = make_in_maps(inputs, gamma)
    res = bass_utils.run_bass_kernel_spmd(
        nc, in_maps, core_ids=list(range(N_CORES)), trace=trace, **kw
    )
    out = np.concatenate([r["o"] for r in res.results], axis=0)
    return out.reshape(B_TOTAL, H, W, C).astype(np.float32, copy=False), res


def kernel(inputs, gamma):
    out, _ = run(inputs, gamma, trace=False)
    return out


# revision 4
# speedup vs baseline: 1.2918x; 1.2918x over previous
"""Channel-attention (CAM) Bass kernel for TRN2, SPMD over 8 NeuronCores.

Computes, for each batch b:
    A   = inputs[b].reshape(HW, C)
    G   = A.T @ A                      (Gram, [C, C])
    S   = softmax(G, axis=-1)
    out = gamma * (A @ S) + A

Sharding: data-parallel over batch. 16 batches / 8 cores = 2 batches per core.

Numerics: residual form
    out = A @ (gamma*S - gamma*I) + (1 + gamma) * A
The (1+gamma)*A term is applied from the exact fp32 copy of A in the DVE
epilogue, so matmul precision only touches the gamma*(S - I) term.  That
lets both matmuls run in fp8e4 with DoubleRow perf mode (2 contraction
rows per PE cell per cycle).

Per-core schedule (per batch):
  - A loaded fp32 as 8 DMA groups of [128, 4, 512]; ALL input-load DMAs
    (both batches) are dispatched before any output DMA so input gets
    queue priority on the shared DMA engines.
  - GpSimd casts each group to fp8 (the raw fp32 tile stays resident for
    the epilogue residual).
  - Gram: fp8 DoubleRow matmuls (2 chunks of 128 rows per MM) into 4 PSUM
    banks; PE transposes of A (fp8, for the attend stationary operand)
    interleaved per group, last 2 groups deferred to cover softmax.
  - Softmax: DVE row-max (negated) -> ScalarE Exp with accum_out row-sum
    -> DVE reciprocal -> scale by gamma -> S'' = (E * gamma*r) - gamma*I
    written as fp8 into pair-layout tiles [128, 2, 512].
  - Attend: per row chunk, 2 DoubleRow MMs (256 channels each).
  - Epilogue: out = psum + (1+gamma)*raw in one DVE scalar_tensor_tensor,
    then DMA out per 2-chunk group.
  - ~36 warmup matmuls at kernel start push the PE HAM clock-gate to
    8/8 (2.4 GHz) before the first Gram work arrives.
"""

import numpy as np

import concourse.bass as bass
import concourse.mybir as mybir
import concourse.tile as tile
from concourse import bacc
from concourse.bass import ds, ts
from concourse.masks import make_identity

P = 128
N_CORES = 8
B_TOTAL = 16
B_PER_CORE = B_TOTAL // N_CORES  # 2
H = 64
W = 64
HW = H * W          # 4096
C = 512
KO = HW // P        # 32 row chunks of A
M = C // P          # 4 channel chunks
NQ = M // 2         # 2 channel-block pairs (DoubleRow)
NG = 8              # DMA groups
KPG = KO // NG      # chunks per group (4)
DEFER_G = 2         # transpose groups deferred past the Gram to cover softmax
N_WARM = 36         # warmup matmuls (~3.6us) to flip HAM to 8/8

F32 = mybir.dt.float32
BF16 = mybir.dt.bfloat16
F8 = mybir.dt.float8e4
AX = mybir.AxisListType
ALU = mybir.AluOpType
ACT_FN = mybir.ActivationFunctionType
DR = mybir.MatmulPerfMode.DoubleRow


def _build_kernel(tc, a_dram, gamma_dram, o_dram):
    nc = tc.nc
    from contextlib import ExitStack

    with ExitStack() as ctx:
        const_pool = ctx.enter_context(tc.tile_pool(name="const", bufs=1))
        a_pool = ctx.enter_context(tc.tile_pool(name="a", bufs=13))
        a8_pool = ctx.enter_context(tc.tile_pool(name="a8", bufs=10))
        at_pool = ctx.enter_context(tc.tile_pool(name="at", bufs=2 * NQ * NG))
        e_pool = ctx.enter_context(tc.tile_pool(name="e", bufs=2))
        s_pool = ctx.enter_context(tc.tile_pool(name="s", bufs=2 * NQ))
        st_pool = ctx.enter_context(tc.tile_pool(name="st", bufs=16))
        o_pool = ctx.enter_context(tc.tile_pool(name="o", bufs=3))
        pg_pool = ctx.enter_context(tc.tile_pool(name="pg", bufs=M, space="PSUM"))
        pt_pool = ctx.enter_context(tc.tile_pool(name="pt", bufs=2, space="PSUM"))
        po_pool = ctx.enter_context(tc.tile_pool(name="po", bufs=2, space="PSUM"))

        # ---- input DMAs first: gets them to the head of every DMA queue ----
        gamma_sb = const_pool.tile([P, 1], F32, tag="gamma")
        nc.sync.dma_start(gamma_sb, gamma_dram)
        raws = [[None] * NG for _ in range(B_PER_CORE)]
        for b in range(B_PER_CORE):
            a_b = a_dram[b].rearrange("(ko p) c -> p ko c", p=P)
            for g in range(NG):
                raw = a_pool.tile([P, KPG, C], F32, tag="a", name="raw")
                nc.sync.dma_start(raw, a_b[:, ts(g, KPG), :])
                raws[b][g] = raw

        # ---- constants ----
        ident8 = const_pool.tile([P, P], F8, tag="ident8")
        make_identity(nc, ident8)
        gamma2_sb = const_pool.tile([P, 1], F32, tag="gamma2")
        nc.vector.tensor_scalar_add(gamma2_sb, gamma_sb, 1.0)
        # identrow[m]: gamma * I placed at columns [128m, 128m+128) of a
        # [128, 512] row block, fp32
        identrow = []
        for m in range(M):
            ir = const_pool.tile([P, C], F32, tag=f"identrow{m}", name="ir")
            nc.gpsimd.memset(ir, 0.0)
            make_identity(nc, ir[:, ts(m, P)], nomemset=True)
            nc.vector.tensor_scalar_mul(ir, ir, gamma_sb)
            identrow.append(ir)

        # ---- PE warmup: ~3.6us of back-to-back matmuls flips HAM to 8/8
        # before the first Gram group lands. ----
        warm_ps = pg_pool.tile([P, P], F32, tag="pg", name="warm")
        for _ in range(N_WARM):
            nc.tensor.matmul(warm_ps, ident8, ident8, start=True, stop=True)

        o_bs = [o_dram[b].rearrange("(u p) c -> p u c", p=P) for b in range(B_PER_CORE)]

        for b in range(B_PER_CORE):
            a8 = []
            at = [[None] * NG for _ in range(NQ)]
            g_ps = [pg_pool.tile([P, C], F32, tag="pg", name="g_ps") for _ in range(M)]

            def do_transposes(g, b=b, a8=a8, at=at):
                for q in range(NQ):
                    atg = at_pool.tile([P, 2, KPG * P], F8, tag="at", name="at")
                    for i in range(2):
                        m = 2 * q + i
                        # fp8 transpose-mode requires output element step 2
                        pt = pt_pool.tile([P, KPG * P, 2], F8, tag="pt", name="pt")
                        for j in range(KPG):
                            nc.tensor.transpose(
                                pt[:, ts(j, P), 0],
                                a8[g][:, j, ts(m, P)],
                                ident8,
                            )
                        nc.scalar.activation(
                            atg[:, i, :], pt[:, :, 0], ACT_FN.Copy, bias=0.0, scale=1.0
                        )
                    at[q][g] = atg

            # Gram accumulation (fp8 DoubleRow, 2 chunks per MM); transposes
            # interleaved per group except the last DEFER_G groups (those
            # cover the softmax latency)
            for g in range(NG):
                a8g = a8_pool.tile([P, KPG, C], F8, tag="a8", name="a8g")
                nc.scalar.activation(a8g, raws[b][g], ACT_FN.Copy, bias=0.0, scale=1.0)
                a8.append(a8g)
                if g < NG - 1:
                    for t in range(KPG // 2):
                        pair = a8g[:, 2 * t : 2 * t + 2, :]
                        for m in range(M):
                            nc.tensor.matmul(
                                g_ps[m],
                                pair[:, :, ts(m, P)],
                                pair,
                                start=(g == 0 and t == 0),
                                stop=False,
                                perf_mode=DR,
                            )
                    if g < NG - DEFER_G:
                        do_transposes(g)
            # last group m-outer: G[m] completes in m order so the softmax
            # chain pipelines per-m right behind the Gram
            gl = NG - 1
            for m in range(M):
                for t in range(KPG // 2):
                    pair = a8[gl][:, 2 * t : 2 * t + 2, :]
                    nc.tensor.matmul(
                        g_ps[m],
                        pair[:, :, ts(m, P)],
                        pair,
                        start=False,
                        stop=(t == KPG // 2 - 1),
                        perf_mode=DR,
                    )

            # Row softmax of G -> S'' = gamma*S - gamma*I, fp8 pair layout.
            s_pair = [
                s_pool.tile([P, 2, C], F8, tag="s", name="s_pair") for _ in range(NQ)
            ]
            for m in range(M):
                negmax = st_pool.tile([P, 1], F32, tag="stat", name="negmax")
                nc.vector.tensor_reduce(
                    negmax, g_ps[m], axis=AX.X, op=ALU.max, negate=True
                )
                e = e_pool.tile([P, C], F32, tag="e", name="e")
                dsum = st_pool.tile([P, 1], F32, tag="stat", name="dsum")
                nc.scalar.activation(
                    e, g_ps[m], ACT_FN.Exp, bias=negmax, scale=1.0, accum_out=dsum
                )
                r = st_pool.tile([P, 1], F32, tag="stat", name="r")
                nc.vector.reciprocal(r, dsum)
                r2 = st_pool.tile([P, 1], F32, tag="stat", name="r2")
                nc.vector.tensor_scalar_mul(r2, r, gamma_sb)
                nc.vector.scalar_tensor_tensor(
                    s_pair[m // 2][:, m % 2, :],
                    e,
                    r2,
                    identrow[m],
                    op0=ALU.mult,
                    op1=ALU.subtract,
                )

            # deferred transposes run on PE while softmax latency drains
            for g in range(NG - DEFER_G, NG):
                do_transposes(g)

            # Attend (A @ S'') + residual epilogue; out-DMA per 2 chunks
            o_sb = None
            for t_i in range(KO):
                g, j = t_i // KPG, t_i % KPG
                o_ps = po_pool.tile([P, C], F32, tag="po", name="o_ps")
                for q in range(NQ):
                    nc.tensor.matmul(
                        o_ps,
                        at[q][g][:, :, ts(j, P)],
                        s_pair[q],
                        start=(q == 0),
                        stop=(q == NQ - 1),
                        perf_mode=DR,
                    )
                if t_i % 2 == 0:
                    o_sb = o_pool.tile([P, 2, C], F32, tag="o", name="o_sb")
                nc.vector.scalar_tensor_tensor(
                    o_sb[:, t_i % 2, :],
                    raws[b][g][:, j, :],
                    gamma2_sb,
                    o_ps,
                    op0=ALU.mult,
                    op1=ALU.add,
                )
                if t_i % 2 == 1:
                    nc.sync.dma_start(o_bs[b][:, t_i - 1 : t_i + 1, :], o_sb)


_NC_CACHE = None


def build():
    global _NC_CACHE
    if _NC_CACHE is not None:
        return _NC_CACHE
    nc = bacc.Bacc(
        "TRN2",
        target_bir_lowering=False,
        debug=False,
        enable_asserts=False,
        num_devices=N_CORES,
    )
    a_dram = nc.dram_tensor("a", [B_PER_CORE, HW, C], F32, kind="ExternalInput").ap()
    gamma_dram = nc.dram_tensor("gamma", [P, 1], F32, kind="ExternalInput").ap()
    o_dram = nc.dram_tensor("o", [B_PER_CORE, HW, C], F32, kind="ExternalOutput").ap()
    with tile.TileContext(nc) as tc:
        _build_kernel(tc, a_dram, gamma_dram, o_dram)
    nc.compile()
    _NC_CACHE = nc
    return nc


def make_in_maps(inputs, gamma):
    x = np.ascontiguousarray(np.asarray(inputs, dtype=np.float32)).reshape(
        B_TOTAL, HW, C
    )
    gb = np.ascontiguousarray(
        np.broadcast_to(np.asarray(gamma, dtype=np.float32).reshape(1, 1), (P, 1))
    )
    return [
        {"a": x[i * B_PER_CORE : (i + 1) * B_PER_CORE], "gamma": gb}
        for i in range(N_CORES)
    ]


def run(inputs, gamma, trace=False, **kw):
    from concourse import bass_utils

    nc = build()
    in_maps = make_in_maps(inputs, gamma)
    res = bass_utils.run_bass_kernel_spmd(
        nc, in_maps, core_ids=list(range(N_CORES)), trace=trace, **kw
    )
    out = np.concatenate([r["o"] for r in res.results], axis=0)
    return out.reshape(B_TOTAL, H, W, C).astype(np.float32, copy=False), res


def kernel(inputs, gamma):
    out, _ = run(inputs, gamma, trace=False)
    return out


# revision 5
# speedup vs baseline: 1.4580x; 1.1286x over previous
"""Channel-attention (CAM) Bass kernel for TRN2, SPMD over 8 NeuronCores.

Computes, for each batch b:
    A   = inputs[b].reshape(HW, C)
    G   = A.T @ A                      (Gram, [C, C])
    S   = softmax(G, axis=-1)
    out = gamma * (A @ S) + A

Sharding: data-parallel over batch. 16 batches / 8 cores = 2 batches per core.

Numerics: residual form
    out = A @ (gamma*S - gamma*I) + (1 + gamma) * A
The (1+gamma)*A term is applied from the exact fp32 copy of A in the DVE
epilogue, so matmul precision only touches the gamma*(S - I) term.  That
lets both matmuls run in fp8e4 with DoubleRow perf mode (2 contraction
rows per PE cell per cycle).

Per-core schedule (per batch):
  - A loaded fp32 as 8 DMA groups of [128, 4, 512]; ALL input-load DMAs
    (both batches) are dispatched before any output DMA so input gets
    queue priority on the shared DMA engines.
  - GpSimd casts each group to fp8 (the raw fp32 tile stays resident for
    the epilogue residual).
  - Gram: fp8 DoubleRow matmuls (2 chunks of 128 rows per MM) into 4 PSUM
    banks; PE transposes of A (fp8, for the attend stationary operand)
    interleaved per group, last 2 groups deferred to cover softmax.
  - Softmax: DVE row-max (negated) -> ScalarE Exp with accum_out row-sum
    -> DVE reciprocal -> scale by gamma -> S'' = (E * gamma*r) - gamma*I
    written as fp8 into pair-layout tiles [128, 2, 512].
  - Attend: per row chunk, 2 DoubleRow MMs (256 channels each).
  - Epilogue: out = psum + (1+gamma)*raw in one DVE scalar_tensor_tensor,
    then DMA out per 2-chunk group.
  - ~36 warmup matmuls at kernel start push the PE HAM clock-gate to
    8/8 (2.4 GHz) before the first Gram work arrives.
"""

import numpy as np

import concourse.bass as bass
import concourse.mybir as mybir
import concourse.tile as tile
from concourse import bacc
from concourse.bass import ds, ts
from concourse.masks import make_identity

P = 128
N_CORES = 8
B_TOTAL = 16
B_PER_CORE = B_TOTAL // N_CORES  # 2
H = 64
W = 64
HW = H * W          # 4096
C = 512
KO = HW // P        # 32 row chunks of A
M = C // P          # 4 channel chunks
NQ = M // 2         # 2 channel-block pairs (DoubleRow)
NG = 8              # DMA groups
KPG = KO // NG      # chunks per group (4)
DEFER_G = 2         # transpose groups deferred past the Gram to cover softmax
N_WARM = 36         # warmup matmuls (~3.6us) to flip HAM to 8/8

F32 = mybir.dt.float32
BF16 = mybir.dt.bfloat16
F8 = mybir.dt.float8e4
AX = mybir.AxisListType
ALU = mybir.AluOpType
ACT_FN = mybir.ActivationFunctionType
DR = mybir.MatmulPerfMode.DoubleRow


def _build_kernel(tc, a_dram, gamma_dram, o_dram):
    nc = tc.nc
    from contextlib import ExitStack

    with ExitStack() as ctx:
        const_pool = ctx.enter_context(tc.tile_pool(name="const", bufs=1))
        a_pool = ctx.enter_context(tc.tile_pool(name="a", bufs=13))
        a8_pool = ctx.enter_context(tc.tile_pool(name="a8", bufs=10))
        at_pool = ctx.enter_context(tc.tile_pool(name="at", bufs=2 * NQ * NG))
        e_pool = ctx.enter_context(tc.tile_pool(name="e", bufs=2))
        s_pool = ctx.enter_context(tc.tile_pool(name="s", bufs=2 * NQ))
        st_pool = ctx.enter_context(tc.tile_pool(name="st", bufs=16))
        o_pool = ctx.enter_context(tc.tile_pool(name="o", bufs=3))
        pg_pool = ctx.enter_context(tc.tile_pool(name="pg", bufs=M, space="PSUM"))
        pt_pool = ctx.enter_context(tc.tile_pool(name="pt", bufs=2, space="PSUM"))
        po_pool = ctx.enter_context(tc.tile_pool(name="po", bufs=2, space="PSUM"))

        # ---- input DMAs first: gets them to the head of every DMA queue ----
        gamma_sb = const_pool.tile([P, 1], F32, tag="gamma")
        nc.sync.dma_start(gamma_sb, gamma_dram)
        raws = [[None] * NG for _ in range(B_PER_CORE)]
        for b in range(B_PER_CORE):
            a_b = a_dram[b].rearrange("(ko p) c -> p ko c", p=P)
            for g in range(NG):
                raw = a_pool.tile([P, KPG, C], F32, tag="a", name="raw")
                nc.sync.dma_start(raw, a_b[:, ts(g, KPG), :])
                raws[b][g] = raw

        # ---- constants ----
        ident8 = const_pool.tile([P, P], F8, tag="ident8")
        make_identity(nc, ident8)
        gamma2_sb = const_pool.tile([P, 1], F32, tag="gamma2")
        nc.vector.tensor_scalar_add(gamma2_sb, gamma_sb, 1.0)
        # identrow[m]: gamma * I placed at columns [128m, 128m+128) of a
        # [128, 512] row block, fp32
        identrow = []
        for m in range(M):
            ir = const_pool.tile([P, C], F32, tag=f"identrow{m}", name="ir")
            nc.gpsimd.memset(ir, 0.0)
            make_identity(nc, ir[:, ts(m, P)], nomemset=True)
            nc.vector.tensor_scalar_mul(ir, ir, gamma_sb)
            identrow.append(ir)

        # ---- PE warmup: ~3.6us of back-to-back matmuls flips HAM to 8/8
        # before the first Gram group lands. ----
        warm_ps = pg_pool.tile([P, P], F32, tag="pg", name="warm")
        for _ in range(N_WARM):
            nc.tensor.matmul(warm_ps, ident8, ident8, start=True, stop=True)

        o_bs = [o_dram[b].rearrange("(u p) c -> p u c", p=P) for b in range(B_PER_CORE)]

        a8s = [[] for _ in range(B_PER_CORE)]
        ats = [[[None] * NG for _ in range(NQ)] for _ in range(B_PER_CORE)]
        g_pss = [None] * B_PER_CORE
        s_pairs = [None] * B_PER_CORE

        def do_transposes(b, g):
            a8, at = a8s[b], ats[b]
            for q in range(NQ):
                atg = at_pool.tile([P, 2, KPG * P], F8, tag="at", name="at")
                for i in range(2):
                    m = 2 * q + i
                    # fp8 transpose-mode requires output element step 2
                    pt = pt_pool.tile([P, KPG * P, 2], F8, tag="pt", name="pt")
                    for j in range(KPG):
                        nc.tensor.transpose(
                            pt[:, ts(j, P), 0],
                            a8[g][:, j, ts(m, P)],
                            ident8,
                        )
                    if i == 0:
                        nc.vector.tensor_copy(out=atg[:, i, :], in_=pt[:, :, 0])
                    else:
                        nc.scalar.activation(
                            atg[:, i, :], pt[:, :, 0], ACT_FN.Copy, bias=0.0, scale=1.0
                        )
                at[q][g] = atg

        def emit_group(b, g):
            """cast + Gram MMs (+ interleaved transposes) for one DMA group."""
            a8g = a8_pool.tile([P, KPG, C], F8, tag="a8", name="a8g")
            nc.scalar.activation(a8g, raws[b][g], ACT_FN.Copy, bias=0.0, scale=1.0)
            a8s[b].append(a8g)
            g_ps = g_pss[b]
            if g < NG - 1:
                for t in range(KPG // 2):
                    pair = a8g[:, 2 * t : 2 * t + 2, :]
                    for m in range(M):
                        nc.tensor.matmul(
                            g_ps[m],
                            pair[:, :, ts(m, P)],
                            pair,
                            start=(g == 0 and t == 0),
                            stop=False,
                            perf_mode=DR,
                        )
                if g < NG - DEFER_G:
                    do_transposes(b, g)
            else:
                # last group m-outer: G[m] completes in m order so the
                # softmax chain pipelines per-m right behind the Gram
                for m in range(M):
                    for t in range(KPG // 2):
                        pair = a8g[:, 2 * t : 2 * t + 2, :]
                        nc.tensor.matmul(
                            g_ps[m],
                            pair[:, :, ts(m, P)],
                            pair,
                            start=False,
                            stop=(t == KPG // 2 - 1),
                            perf_mode=DR,
                        )

        def emit_softmax(b):
            """Row softmax of G -> S'' = gamma*S - gamma*I, fp8 pair layout;
            then the deferred transposes (PE work covering softmax latency)."""
            s_pair = [
                s_pool.tile([P, 2, C], F8, tag="s", name="s_pair") for _ in range(NQ)
            ]
            s_pairs[b] = s_pair
            for m in range(M):
                negmax = st_pool.tile([P, 1], F32, tag="stat", name="negmax")
                nc.vector.tensor_reduce(
                    negmax, g_pss[b][m], axis=AX.X, op=ALU.max, negate=True
                )
                e = e_pool.tile([P, C], F32, tag="e", name="e")
                dsum = st_pool.tile([P, 1], F32, tag="stat", name="dsum")
                nc.scalar.activation(
                    e, g_pss[b][m], ACT_FN.Exp, bias=negmax, scale=1.0, accum_out=dsum
                )
                r = st_pool.tile([P, 1], F32, tag="stat", name="r")
                nc.vector.reciprocal(r, dsum)
                r2 = st_pool.tile([P, 1], F32, tag="stat", name="r2")
                nc.vector.tensor_scalar_mul(r2, r, gamma_sb)
                nc.vector.scalar_tensor_tensor(
                    s_pair[m // 2][:, m % 2, :],
                    e,
                    r2,
                    identrow[m],
                    op0=ALU.mult,
                    op1=ALU.subtract,
                )
            for g in range(NG - DEFER_G, NG):
                do_transposes(b, g)

        def emit_attend_chunk(b, t_i, o_sb_box):
            """Attend MMs + residual epilogue for one row chunk; out-DMA per
            4 chunks."""
            g, j = t_i // KPG, t_i % KPG
            o_ps = po_pool.tile([P, C], F32, tag="po", name="o_ps")
            for q in range(NQ):
                nc.tensor.matmul(
                    o_ps,
                    ats[b][q][g][:, :, ts(j, P)],
                    s_pairs[b][q],
                    start=(q == 0),
                    stop=(q == NQ - 1),
                    perf_mode=DR,
                )
            if t_i % KPG == 0:
                o_sb_box[0] = o_pool.tile([P, KPG, C], F32, tag="o", name="o_sb")
            nc.vector.scalar_tensor_tensor(
                o_sb_box[0][:, t_i % KPG, :],
                raws[b][g][:, j, :],
                gamma2_sb,
                o_ps,
                op0=ALU.mult,
                op1=ALU.add,
            )
            if t_i % KPG == KPG - 1:
                nc.sync.dma_start(
                    o_bs[b][:, t_i - (KPG - 1) : t_i + 1, :], o_sb_box[0]
                )

        # ---- batch 0 load/gram/softmax ----
        g_pss[0] = [pg_pool.tile([P, C], F32, tag="pg", name="g_ps") for _ in range(M)]
        for g in range(NG):
            emit_group(0, g)
        emit_softmax(0)

        # ---- batch 0 attend, with batch 1's group pipeline interleaved so
        # the PE queue alternates between attend-0 chunks and gram-1 work ----
        g_pss[1] = [pg_pool.tile([P, C], F32, tag="pg", name="g_ps") for _ in range(M)]
        o_sb_box = [None]
        for t_i in range(KO):
            emit_attend_chunk(0, t_i, o_sb_box)
            if t_i % KPG == KPG - 1:
                emit_group(1, t_i // KPG)
        emit_softmax(1)

        # ---- batch 1 attend ----
        o_sb_box = [None]
        for t_i in range(KO):
            emit_attend_chunk(1, t_i, o_sb_box)


_NC_CACHE = None


def build():
    global _NC_CACHE
    if _NC_CACHE is not None:
        return _NC_CACHE
    nc = bacc.Bacc(
        "TRN2",
        target_bir_lowering=False,
        debug=False,
        enable_asserts=False,
        num_devices=N_CORES,
    )
    a_dram = nc.dram_tensor("a", [B_PER_CORE, HW, C], F32, kind="ExternalInput").ap()
    gamma_dram = nc.dram_tensor("gamma", [P, 1], F32, kind="ExternalInput").ap()
    o_dram = nc.dram_tensor("o", [B_PER_CORE, HW, C], F32, kind="ExternalOutput").ap()
    with tile.TileContext(nc) as tc:
        _build_kernel(tc, a_dram, gamma_dram, o_dram)
    nc.compile()
    _NC_CACHE = nc
    return nc


def make_in_maps(inputs, gamma):
    x = np.ascontiguousarray(np.asarray(inputs, dtype=np.float32)).reshape(
        B_TOTAL, HW, C
    )
    gb = np.ascontiguousarray(
        np.broadcast_to(np.asarray(gamma, dtype=np.float32).reshape(1, 1), (P, 1))
    )
    return [
        {"a": x[i * B_PER_CORE : (i + 1) * B_PER_CORE], "gamma": gb}
        for i in range(N_CORES)
    ]


def run(inputs, gamma, trace=False, **kw):
    from concourse import bass_utils

    nc = build()
    in_maps = make_in_maps(inputs, gamma)
    res = bass_utils.run_bass_kernel_spmd(
        nc, in_maps, core_ids=list(range(N_CORES)), trace=trace, **kw
    )
    out = np.concatenate([r["o"] for r in res.results], axis=0)
    return out.reshape(B_TOTAL, H, W, C).astype(np.float32, copy=False), res


def kernel(inputs, gamma):
    out, _ = run(inputs, gamma, trace=False)
    return out


# revision 11
# speedup vs baseline: 1.4803x; 1.0153x over previous
"""Channel-attention (CAM) Bass kernel for TRN2, SPMD over 8 NeuronCores.

Computes, for each batch b:
    A   = inputs[b].reshape(HW, C)
    G   = A.T @ A                      (Gram, [C, C])
    S   = softmax(G, axis=-1)
    out = gamma * (A @ S) + A

Sharding: data-parallel over batch. 16 batches / 8 cores = 2 batches per core.

Numerics: residual form
    out = A @ (gamma*S - gamma*I) + (1 + gamma) * A
The (1+gamma)*A term is applied from the exact fp32 copy of A in the DVE
epilogue, so matmul precision only touches the gamma*(S - I) term.  That
lets both matmuls run in fp8e4 with DoubleRow perf mode (2 contraction
rows per PE cell per cycle).

Per-core schedule (per batch):
  - A loaded fp32 as 8 DMA groups of [128, 4, 512]; ALL input-load DMAs
    (both batches) are dispatched before any output DMA so input gets
    queue priority on the shared DMA engines.
  - GpSimd casts each group to fp8 (the raw fp32 tile stays resident for
    the epilogue residual).
  - Gram: fp8 DoubleRow matmuls (2 chunks of 128 rows per MM) into 4 PSUM
    banks; PE transposes of A (fp8, for the attend stationary operand)
    interleaved per group, last 2 groups deferred to cover softmax.
  - Softmax: DVE row-max (negated) -> ScalarE Exp with accum_out row-sum
    -> DVE reciprocal -> scale by gamma -> S'' = (E * gamma*r) - gamma*I
    written as fp8 into pair-layout tiles [128, 2, 512].
  - Attend: per row chunk, 2 DoubleRow MMs (256 channels each).
  - Epilogue: out = psum + (1+gamma)*raw in one DVE scalar_tensor_tensor,
    then DMA out per 2-chunk group.
  - ~36 warmup matmuls at kernel start push the PE HAM clock-gate to
    8/8 (2.4 GHz) before the first Gram work arrives.
"""

import numpy as np

import concourse.bass as bass
import concourse.mybir as mybir
import concourse.tile as tile
from concourse import bacc
from concourse.bass import ds, ts
from concourse.masks import make_identity

P = 128
N_CORES = 8
B_TOTAL = 16
B_PER_CORE = B_TOTAL // N_CORES  # 2
H = 64
W = 64
HW = H * W          # 4096
C = 512
KO = HW // P        # 32 row chunks of A
M = C // P          # 4 channel chunks
NQ = M // 2         # 2 channel-block pairs (DoubleRow)
NG = 8              # DMA groups
KPG = KO // NG      # chunks per group (4)
DEFER_G = 2         # transpose groups deferred past the Gram to cover softmax
N_WARM = 36         # warmup matmuls (~3.6us) to flip HAM to 8/8

F32 = mybir.dt.float32
BF16 = mybir.dt.bfloat16
F8 = mybir.dt.float8e4
AX = mybir.AxisListType
ALU = mybir.AluOpType
ACT_FN = mybir.ActivationFunctionType
DR = mybir.MatmulPerfMode.DoubleRow


def _build_kernel(tc, a_dram, gamma_dram, o_dram):
    nc = tc.nc
    from contextlib import ExitStack

    with ExitStack() as ctx:
        const_pool = ctx.enter_context(tc.tile_pool(name="const", bufs=1))
        a_pool = ctx.enter_context(tc.tile_pool(name="a", bufs=13))
        a8_pool = ctx.enter_context(tc.tile_pool(name="a8", bufs=10))
        at_pool = ctx.enter_context(tc.tile_pool(name="at", bufs=2 * NQ * NG))
        e_pool = ctx.enter_context(tc.tile_pool(name="e", bufs=2))
        s_pool = ctx.enter_context(tc.tile_pool(name="s", bufs=2 * NQ))
        st_pool = ctx.enter_context(tc.tile_pool(name="st", bufs=16))
        o_pool = ctx.enter_context(tc.tile_pool(name="o", bufs=3))
        pg_pool = ctx.enter_context(tc.tile_pool(name="pg", bufs=M, space="PSUM"))
        pt_pool = ctx.enter_context(tc.tile_pool(name="pt", bufs=2, space="PSUM"))
        po_pool = ctx.enter_context(tc.tile_pool(name="po", bufs=2, space="PSUM"))

        # ---- input DMAs first: gets them to the head of every DMA queue ----
        gamma_sb = const_pool.tile([P, 1], F32, tag="gamma")
        nc.sync.dma_start(gamma_sb, gamma_dram)
        raws = [[None] * NG for _ in range(B_PER_CORE)]
        for b in range(B_PER_CORE):
            a_b = a_dram[b].rearrange("(ko p) c -> p ko c", p=P)
            for g in range(NG):
                raw = a_pool.tile([P, KPG, C], F32, tag="a", name="raw")
                nc.sync.dma_start(raw, a_b[:, ts(g, KPG), :])
                raws[b][g] = raw

        # ---- constants ----
        ident8 = const_pool.tile([P, P], F8, tag="ident8")
        make_identity(nc, ident8)
        gamma2_sb = const_pool.tile([P, 1], F32, tag="gamma2")
        nc.vector.tensor_scalar_add(gamma2_sb, gamma_sb, 1.0)
        # identrow[m]: gamma * I placed at columns [128m, 128m+128) of a
        # [128, 512] row block, fp32
        identrow = []
        for m in range(M):
            ir = const_pool.tile([P, C], F32, tag=f"identrow{m}", name="ir")
            nc.gpsimd.memset(ir, 0.0)
            make_identity(nc, ir[:, ts(m, P)], nomemset=True)
            nc.vector.tensor_scalar_mul(ir, ir, gamma_sb)
            identrow.append(ir)

        # ---- PE warmup: ~3.6us of back-to-back matmuls flips HAM to 8/8
        # before the first Gram group lands. ----
        warm_ps = pg_pool.tile([P, P], F32, tag="pg", name="warm")
        for _ in range(N_WARM):
            nc.tensor.matmul(warm_ps, ident8, ident8, start=True, stop=True)

        o_bs = [o_dram[b].rearrange("(u p) c -> p u c", p=P) for b in range(B_PER_CORE)]

        a8s = [[] for _ in range(B_PER_CORE)]
        ats = [[[None] * NG for _ in range(NQ)] for _ in range(B_PER_CORE)]
        g_pss = [None] * B_PER_CORE
        s_pairs = [None] * B_PER_CORE

        def do_transposes(b, g):
            # batch 0's drains go to DVE (idle during the gram-0 phase, while
            # ScalarE is busy casting); batch 1's go to ScalarE (idle during
            # the attend-0 window, while DVE paces the epilogue).
            a8, at = a8s[b], ats[b]
            for q in range(NQ):
                atg = at_pool.tile([P, 2, KPG * P], F8, tag="at", name="at")
                # fp8 transpose-mode requires output element step 2; both
                # i-slabs share one pt bank so the pair drains in one copy.
                pt = pt_pool.tile([P, 2, KPG * P, 2], F8, tag="pt", name="pt")
                for i in range(2):
                    m = 2 * q + i
                    for j in range(KPG):
                        nc.tensor.transpose(
                            pt[:, i, ts(j, P), 0],
                            a8[g][:, j, ts(m, P)],
                            ident8,
                        )
                if b == 0:
                    nc.vector.tensor_copy(out=atg, in_=pt[:, :, :, 0])
                else:
                    nc.scalar.activation(
                        atg, pt[:, :, :, 0], ACT_FN.Copy, bias=0.0, scale=1.0
                    )
                at[q][g] = atg

        def emit_group(b, g):
            """cast + Gram MMs (+ interleaved transposes) for one DMA group."""
            a8g = a8_pool.tile([P, KPG, C], F8, tag="a8", name="a8g")
            nc.scalar.activation(a8g, raws[b][g], ACT_FN.Copy, bias=0.0, scale=1.0)
            a8s[b].append(a8g)
            g_ps = g_pss[b]
            if g < NG - 2:
                for t in range(KPG // 2):
                    pair = a8g[:, 2 * t : 2 * t + 2, :]
                    for m in range(M):
                        nc.tensor.matmul(
                            g_ps[m],
                            pair[:, :, ts(m, P)],
                            pair,
                            start=(g == 0 and t == 0),
                            stop=False,
                            perf_mode=DR,
                        )
                if g < NG - DEFER_G:
                    do_transposes(b, g)
            elif g == NG - 1:
                # last two groups m-outer: G[m] completes in m order so the
                # softmax chain pipelines per-m behind the Gram tail
                for m in range(M):
                    for gg in (NG - 2, NG - 1):
                        for t in range(KPG // 2):
                            pair = a8s[b][gg][:, 2 * t : 2 * t + 2, :]
                            nc.tensor.matmul(
                                g_ps[m],
                                pair[:, :, ts(m, P)],
                                pair,
                                start=False,
                                stop=(gg == NG - 1 and t == KPG // 2 - 1),
                                perf_mode=DR,
                            )

        def emit_softmax(b):
            """Row softmax of G -> S'' = gamma*S - gamma*I, fp8 pair layout;
            then the deferred transposes (PE work covering softmax latency)."""
            s_pair = [
                s_pool.tile([P, 2, C], F8, tag="s", name="s_pair") for _ in range(NQ)
            ]
            s_pairs[b] = s_pair
            for m in range(M):
                negmax = st_pool.tile([P, 1], F32, tag="stat", name="negmax")
                nc.vector.tensor_reduce(
                    negmax, g_pss[b][m], axis=AX.X, op=ALU.max, negate=True
                )
                e = e_pool.tile([P, C], F32, tag="e", name="e")
                dsum = st_pool.tile([P, 1], F32, tag="stat", name="dsum")
                nc.scalar.activation(
                    e, g_pss[b][m], ACT_FN.Exp, bias=negmax, scale=1.0, accum_out=dsum
                )
                r = st_pool.tile([P, 1], F32, tag="stat", name="r")
                nc.vector.reciprocal(r, dsum)
                r2 = st_pool.tile([P, 1], F32, tag="stat", name="r2")
                nc.vector.tensor_scalar_mul(r2, r, gamma_sb)
                nc.vector.scalar_tensor_tensor(
                    s_pair[m // 2][:, m % 2, :],
                    e,
                    r2,
                    identrow[m],
                    op0=ALU.mult,
                    op1=ALU.subtract,
                )
            for g in range(NG - DEFER_G, NG):
                do_transposes(b, g)

        def emit_attend_chunk(b, t_i, o_sb_box):
            """Attend MMs + residual epilogue for one row chunk; out-DMA per
            4 chunks."""
            g, j = t_i // KPG, t_i % KPG
            o_ps = po_pool.tile([P, C], F32, tag="po", name="o_ps")
            for q in range(NQ):
                nc.tensor.matmul(
                    o_ps,
                    ats[b][q][g][:, :, ts(j, P)],
                    s_pairs[b][q],
                    start=(q == 0),
                    stop=(q == NQ - 1),
                    perf_mode=DR,
                )
            if t_i % KPG == 0:
                o_sb_box[0] = o_pool.tile([P, KPG, C], F32, tag="o", name="o_sb")
            nc.vector.scalar_tensor_tensor(
                o_sb_box[0][:, t_i % KPG, :],
                raws[b][g][:, j, :],
                gamma2_sb,
                o_ps,
                op0=ALU.mult,
                op1=ALU.add,
            )
            if t_i % KPG == KPG - 1:
                nc.sync.dma_start(
                    o_bs[b][:, t_i - (KPG - 1) : t_i + 1, :], o_sb_box[0]
                )

        # ---- batch 0 load/gram/softmax ----
        g_pss[0] = [pg_pool.tile([P, C], F32, tag="pg", name="g_ps") for _ in range(M)]
        for g in range(NG):
            emit_group(0, g)
        emit_softmax(0)

        # ---- batch 0 attend, with batch 1's group pipeline interleaved so
        # the PE queue alternates between attend-0 chunks and gram-1 work ----
        g_pss[1] = [pg_pool.tile([P, C], F32, tag="pg", name="g_ps") for _ in range(M)]
        o_sb_box = [None]
        for t_i in range(KO):
            emit_attend_chunk(0, t_i, o_sb_box)
            if t_i % KPG == KPG - 1:
                emit_group(1, t_i // KPG)
        emit_softmax(1)

        # ---- batch 1 attend ----
        o_sb_box = [None]
        for t_i in range(KO):
            emit_attend_chunk(1, t_i, o_sb_box)


_NC_CACHE = None


def build():
    global _NC_CACHE
    if _NC_CACHE is not None:
        return _NC_CACHE
    nc = bacc.Bacc(
        "TRN2",
        target_bir_lowering=False,
        debug=False,
        enable_asserts=False,
        num_devices=N_CORES,
    )
    a_dram = nc.dram_tensor("a", [B_PER_CORE, HW, C], F32, kind="ExternalInput").ap()
    gamma_dram = nc.dram_tensor("gamma", [P, 1], F32, kind="ExternalInput").ap()
    o_dram = nc.dram_tensor("o", [B_PER_CORE, HW, C], F32, kind="ExternalOutput").ap()
    with tile.TileContext(nc) as tc:
        _build_kernel(tc, a_dram, gamma_dram, o_dram)
    nc.compile()
    _NC_CACHE = nc
    return nc


def make_in_maps(inputs, gamma):
    x = np.ascontiguousarray(np.asarray(inputs, dtype=np.float32)).reshape(
        B_TOTAL, HW, C
    )
    gb = np.ascontiguousarray(
        np.broadcast_to(np.asarray(gamma, dtype=np.float32).reshape(1, 1), (P, 1))
    )
    return [
        {"a": x[i * B_PER_CORE : (i + 1) * B_PER_CORE], "gamma": gb}
        for i in range(N_CORES)
    ]


def run(inputs, gamma, trace=False, **kw):
    from concourse import bass_utils

    nc = build()
    in_maps = make_in_maps(inputs, gamma)
    res = bass_utils.run_bass_kernel_spmd(
        nc, in_maps, core_ids=list(range(N_CORES)), trace=trace, **kw
    )
    out = np.concatenate([r["o"] for r in res.results], axis=0)
    return out.reshape(B_TOTAL, H, W, C).astype(np.float32, copy=False), res


def kernel(inputs, gamma):
    out, _ = run(inputs, gamma, trace=False)
    return out
